# revision 1
# baseline (speedup 1.0000x reference)
"""Causal self-attention (B=2, N=2048, D=768, H=12) on 8 Trainium2 NeuronCores.

Sharding: data-parallel over batch (2) x tensor-parallel over head groups (4),
3 heads per core. Each core computes, for its (batch, head-group):
  GEMM1: kT/qT (transposed) and v (natural) projections from xT,
  scores^T = k @ q^T per head, exp on ScalarE (fp16 out),
  AV with a ones-augmented V giving unnormalized sa + row sums,
  normalize, GEMM2 row-parallel -> yT partial (fp16).
All matmul operands are fp16 (fp32 PSUM accumulate). Host shards inputs, sums
the 4 per-batch partials (the "all-reduce"), and adds the output bias fold
(bproj + bkqv_v @ Wproj - exact because softmax rows sum to 1).

v2 changes vs baseline:
  - inputs packed on host so each DMA has >=1KB lines and xT lands
    i-slice-major: GEMM1 starts ~8.5us instead of ~20us
  - PE warmup burst sized to bridge exactly until the first GEMM1 data
    lands (keeps the HAM clock-gate warm into the kernel body)
  - strips (scores+exp) interleaved at instruction granularity with
    independent PE work (v-proj, AV, GEMM2), paced to the Scalar engine's
    exp throughput, so the in-order PE queue never stalls (no HAM
    re-throttle mid-kernel)
  - AV softmax-normalize chain moved off PSUM: one DVE evac frees the
    PSUM bank, then recip (DVE) -> partition broadcast (GpSimd) ->
    multiply (GpSimd) run decoupled
  - GEMM2 contracts heads 0+1 in one 128-partition matmul (saT packed)
  - yT partials stored fp16 (halves output DMA)

Self-contained: hardcodes all shapes; no sibling imports.
"""

import os

import numpy as np

B, N, D = 2, 2048, 768
H, HD = 12, 64
HPC = 3           # heads per core
NG = 4            # head groups
NCORES = 8
P = 128
NJ = N // P       # 16 j-chunks (keys) per head
NISL = 4          # 512-query i-slices

_compiled = None  # cached compiled Bass module
last_exec_time_ns = None
last_results = None

N_WARMUP = 15     # 512-wide dummy matmuls bridging boot -> first GEMM1


def _build():
    import concourse.tile as tile
    import concourse.mybir as mybir
    from concourse import bacc

    f32 = mybir.dt.float32
    f16 = mybir.dt.float16
    ADD = mybir.AluOpType.add
    MULT = mybir.AluOpType.mult
    EXP = mybir.ActivationFunctionType.Exp

    nc = bacc.Bacc(
        "TRN2", target_bir_lowering=False, debug=False, num_devices=NCORES
    )

    # packed DRAM layouts (see _host_prep)
    xT_d = nc.dram_tensor("xTp", [NISL, P, 6 * 512], f16, kind="ExternalInput").ap()
    wkq_d = nc.dram_tensor("wkqp", [3, P, 6 * 128], f16, kind="ExternalInput").ap()
    wv_d = nc.dram_tensor("wvp", [P, 6 * 192], f16, kind="ExternalInput").ap()
    wp01_d = nc.dram_tensor("wp01", [P, D], f16, kind="ExternalInput").ap()
    wp2_d = nc.dram_tensor("wp2", [64, D], f16, kind="ExternalInput").ap()
    bkq_d = nc.dram_tensor("bkq", [P, 4], f32, kind="ExternalInput").ap()
    ident_d = nc.dram_tensor("ident", [P, P], f16, kind="ExternalInput").ap()
    btri_d = nc.dram_tensor("btri", [P, P], f16, kind="ExternalInput").ap()
    yT_d = nc.dram_tensor("yT", [6, P, N], f16, kind="ExternalOutput").ap()

    xT_v = xT_d.rearrange("i p f -> p i f")      # [128, 4, 3072]
    wkq_v = wkq_d.rearrange("c p f -> p c f")    # [128, 3, 768]
    yT_v = yT_d.rearrange("o p f -> p o f")      # [128, 6, 2048]

    with tile.TileContext(nc) as tc:
        import contextlib

        ctx = contextlib.ExitStack()
        with ctx:
            const = ctx.enter_context(tc.tile_pool(name="const", bufs=1))
            big = ctx.enter_context(tc.tile_pool(name="bigbufs", bufs=1))
            work = ctx.enter_context(tc.tile_pool(name="work", bufs=3))
            ypool = ctx.enter_context(tc.tile_pool(name="ypool", bufs=3))
            # [128,512] f32 = 1 bank: sc triple-buffered (3) + gemm (2)
            # + av triple-buffered (3) = 8 banks
            psum_sc = ctx.enter_context(
                tc.tile_pool(name="psum_sc", bufs=3, space="PSUM")
            )
            psum_gemm = ctx.enter_context(
                tc.tile_pool(name="psum_gemm", bufs=2, space="PSUM")
            )
            psum_av = ctx.enter_context(
                tc.tile_pool(name="psum_av", bufs=3, space="PSUM")
            )

            # ---- SBUF tiles ----
            bkq_t = const.tile([P, 4], f32, name="bkq_t")
            wkq_ts = [const.tile([P, 768], f16, name=f"wkq_t{c}") for c in range(3)]
            xT_ts = [big.tile([P, 6 * 512], f16, name=f"xT_t{i}") for i in range(NISL)]
            ident_t = const.tile([P, P], f16, name="ident_t")
            btri_t = const.tile([P, P], f16, name="btri_t")
            wv_t = const.tile([P, 6 * 192], f16, name="wv_t")
            wp01_t = const.tile([P, D], f16, name="wp01_t")
            wp2_t = const.tile([64, D], f16, name="wp2_t")

            # DMA issue order == priority order (first-needed first)
            nc.sync.dma_start(bkq_t[:], bkq_d)
            nc.sync.dma_start(wkq_ts[0][:], wkq_v[:, 0, :])
            nc.sync.dma_start(xT_ts[0][:], xT_v[:, 0, :])
            nc.sync.dma_start(wkq_ts[1][:], wkq_v[:, 1, :])
            nc.sync.dma_start(xT_ts[1][:], xT_v[:, 1, :])
            nc.sync.dma_start(wkq_ts[2][:], wkq_v[:, 2, :])
            nc.sync.dma_start(ident_t[:], ident_d)
            nc.sync.dma_start(btri_t[:], btri_d)
            nc.sync.dma_start(xT_ts[2][:], xT_v[:, 2, :])
            nc.sync.dma_start(wv_t[:], wv_d)
            nc.sync.dma_start(xT_ts[3][:], xT_v[:, 3, :])
            nc.sync.dma_start(wp01_t[:], wp01_d)
            nc.sync.dma_start(wp2_t[:], wp2_d)

            # PE warmup on a zeroed scratch while the first inputs land
            wscr = const.tile([P, 512], f16, name="wscr")
            nc.vector.memset(wscr[:], 0.0)
            for _ in range(N_WARMUP):
                wps = psum_gemm.tile([P, 512], f32, tag="ps512", name="wps")
                nc.tensor.matmul(
                    wps[:, 0:512], wscr[:, 0:128], wscr[:], start=True, stop=True
                )

            kT0 = big.tile([P, N], f16, name="kT0")
            qT0 = big.tile([P, N], f16, name="qT0")
            kT1 = big.tile([64, N], f16, name="kT1")
            qT1 = big.tile([64, N], f16, name="qT1")
            q2st = big.tile([P, N], f16, name="q2st")
            kqT = [(kT0, qT0), (kT1, qT1)]
            vaug = big.tile([P, NJ, HPC, 65], f16, name="vaug")
            nc.vector.memset(vaug[:, :, :, 64:65], 1.0)
            # saT: heads 0+1 packed on 128 partitions; head 2 separate
            saT01s = [big.tile([P, 512], f16, name=f"saT01_{i}") for i in range(4)]
            saT2s = [big.tile([64, 512], f16, name=f"saT2_{i}") for i in range(4)]

            # ---- GEMM1 k/q: psum tile per (isl, ci) accumulated over dc ----
            def emit_gemm1_kq(isl, ci):
                ps = psum_gemm.tile([P, 512], f32, tag="ps512", name="ps_kq")
                for dc in range(6):
                    nc.tensor.matmul(
                        ps[:, 0:512],
                        wkq_ts[ci][:, 128 * dc : 128 * dc + 128],
                        xT_ts[isl][:, 512 * dc : 512 * dc + 512],
                        start=(dc == 0),
                        stop=(dc == 5),
                    )
                sl = slice(512 * isl, 512 * isl + 512)
                if ci < 2:
                    dst = kT0 if ci == 0 else qT0
                    nc.vector.tensor_scalar(
                        dst[:, sl], ps[:, 0:512], bkq_t[:, ci : ci + 1], None, op0=ADD
                    )
                else:
                    nc.vector.tensor_scalar(
                        kT1[0:64, sl], ps[0:64, 0:512], bkq_t[0:64, 2:3], None, op0=ADD
                    )
                    nc.vector.tensor_scalar(
                        q2st[64:128, sl], ps[64:128, 0:512],
                        bkq_t[64:128, 3:4], None, op0=ADD,
                    )
                    nc.sync.dma_start(qT1[0:64, sl], q2st[64:128, sl])

            # ---- GEMM1 v: one psum tile per 128-query chunk ----
            def emit_gemm1_v(ic):
                ps = psum_gemm.tile([P, 512], f32, tag="ps512", name="ps_v")
                isl, k = divmod(ic, 4)
                for dc in range(6):
                    nc.tensor.matmul(
                        ps[:, 0:192],
                        xT_ts[isl][:, 512 * dc + 128 * k : 512 * dc + 128 * k + 128],
                        wv_t[:, 192 * dc : 192 * dc + 192],
                        start=(dc == 0),
                        stop=(dc == 5),
                    )
                nc.vector.tensor_copy(
                    out=vaug[:, ic, :, 0:64],
                    in_=ps[:, 0:192].rearrange("p (h d) -> p h d", h=HPC),
                )

            # ---- strips: scoresT + causal mask + exp ----
            all_strips = [[None] * NJ for _ in range(HPC)]

            def emit_strip(h, jc):
                if h < 2:
                    cc, pb = 0, 64 * h
                else:
                    cc, pb = 1, 0
                kTc, qTc = kqT[cc]
                i0 = 128 * jc
                W = N - i0
                strip = work.tile(
                    [P, W], f16, tag=f"expT{jc}", bufs=3, name=f"expT{jc}"
                )
                for s0 in range(0, W, 512):
                    sw = min(512, W - s0)
                    ps = psum_sc.tile([P, 512], f32, tag="sc", name="ps_s")
                    chained = s0 == 0
                    nc.tensor.matmul(
                        ps[:, 0:sw],
                        kTc[pb : pb + 64, i0 : i0 + 128],
                        qTc[pb : pb + 64, i0 + s0 : i0 + s0 + sw],
                        start=True,
                        stop=(not chained),
                    )
                    if chained:
                        # causal mask: accumulate -30000 above the diagonal
                        nc.tensor.matmul(
                            ps[:, 0:128], ident_t[:], btri_t[:],
                            start=False, stop=True,
                        )
                    nc.scalar.activation(
                        strip[:, s0 : s0 + sw], ps[:, 0:sw], EXP, scale=0.125
                    )
                all_strips[h][jc] = strip

            # ---- AV + normalize ----
            def emit_av(h, iseg):
                strips = all_strips[h]
                ps2 = psum_av.tile([65, 512], f32, tag="av", name="ps2")
                jmax = 4 * iseg + 3
                for jc in range(jmax + 1):
                    off = 512 * iseg - 128 * jc
                    lo = max(0, off)
                    w = 512 - (lo - off)
                    nc.tensor.matmul(
                        ps2[0:65, 512 - w : 512],
                        vaug[:, jc, h, :],
                        strips[jc][:, lo : lo + w],
                        start=(jc == 0),
                        stop=(jc == jmax),
                    )
                # evacuate PSUM immediately; normalize off-PSUM. GpSimd runs
                # ONLY partition_broadcast (mixing op types forces library
                # reloads that stall the whole pipeline ~20us each).
                uav = work.tile([65, 512], f32, tag="uav", bufs=3, name="uav")
                nc.vector.tensor_copy(out=uav[:], in_=ps2[0:65, :])
                srow = work.tile([1, 512], f32, tag="srow", bufs=2, name="srow")
                nc.vector.tensor_copy(out=srow[:], in_=uav[64:65, :])
                rrow = work.tile([1, 512], f32, tag="rrow", bufs=2, name="rrow")
                nc.vector.reciprocal_approx_fast(out=rrow[:], in_=srow[:])
                rbc = work.tile([64, 512], f32, tag="rbc", bufs=2, name="rbc")
                nc.gpsimd.partition_broadcast(rbc[:], rrow[:])
                if h == 0:
                    nc.vector.tensor_tensor(
                        saT01s[iseg][0:64, :], uav[0:64, :], rbc[:], MULT
                    )
                elif h == 1:
                    st1 = work.tile([64, 512], f16, tag="st1", bufs=2, name="st1")
                    nc.vector.tensor_tensor(st1[:], uav[0:64, :], rbc[:], MULT)
                    nc.sync.dma_start(saT01s[iseg][64:128, :], st1[:])
                else:
                    nc.vector.tensor_tensor(
                        saT2s[iseg][:], uav[0:64, :], rbc[:], MULT
                    )

            # ---- GEMM2: heads 0+1 contract-128, head 2 contract-64 ----
            def emit_gemm2(isl):
                for oc in range(6):
                    ps = psum_gemm.tile([P, 512], f32, tag="ps512", name="ps_y")
                    nc.tensor.matmul(
                        ps[:, 0:512],
                        wp01_t[:, 128 * oc : 128 * oc + 128],
                        saT01s[isl][:],
                        start=True,
                        stop=False,
                    )
                    nc.tensor.matmul(
                        ps[:, 0:512],
                        wp2_t[:, 128 * oc : 128 * oc + 128],
                        saT2s[isl][:],
                        start=False,
                        stop=True,
                    )
                    yst = ypool.tile([P, 512], f16, tag="yst", name="yst")
                    nc.vector.tensor_copy(out=yst[:], in_=ps[:, 0:512])
                    nc.sync.dma_start(
                        yT_v[:, oc, 512 * isl : 512 * isl + 512], yst[:]
                    )

            # ---- emission schedule ----
            # Phase A: all k/q projections. ci2 must precede the first h2
            # strip in the PE FIFO (in-order queue: a consumer ahead of its
            # producer deadlocks), so it is not eligible as pacing filler.
            for isl in range(NISL):
                emit_gemm1_kq(isl, 0)
                emit_gemm1_kq(isl, 1)
            for isl in range(NISL):
                emit_gemm1_kq(isl, 2)

            # Phase B: strips paced against ACT, with independent PE work
            # (ci2 projections, v projections, AV groups, GEMM2) as filler.
            # Costs in ns for the pacing model.
            def strip_pe_cost(W):
                return W / 2.4 + 370

            def strip_act_cost(W):
                return 0.85 * W + 270 * ((W + 511) // 512)

            fillers = []  # (ready_act_time, pe_cost, emit_fn)
            for ic in range(16):
                fillers.append((0.0, 560, lambda ic=ic: emit_gemm1_v(ic)))

            pe_t = 0.0    # PE-busy time emitted so far (phase B origin)
            act_t = 0.0   # ACT-busy time emitted so far
            SLACK = 4000.0

            def pop_fillers(budget):
                nonlocal pe_t
                spent = 0.0
                while fillers and spent < budget:
                    ready, cost, fn = fillers[0]
                    if ready > pe_t:
                        break
                    fillers.pop(0)
                    fn()
                    pe_t += cost
                    spent += cost

            strip_order = []
            for g in range(4):
                for h in range(HPC):
                    for jc in range(4 * g, 4 * g + 4):
                        strip_order.append((h, jc, g))

            av_done = 0
            for h, jc, g in strip_order:
                W = N - 128 * jc
                emit_strip(h, jc)
                pe_t += strip_pe_cost(W)
                act_t += strip_act_cost(W)
                if jc == 4 * g + 3:
                    # strips for (h, group g) complete: AV group becomes
                    # available once ACT has drained through them
                    fillers.append(
                        (act_t + SLACK, 215 * (4 * g + 1) + 590,
                         lambda h=h, g=g: emit_av(h, g))
                    )
                    if h == HPC - 1:
                        # extra delay: GEMM2 g needs head 2's normalize
                        # chain (~3us after the AV matmuls) to finish
                        fillers.append(
                            (act_t + SLACK + 6000, 2600,
                             lambda g=g: emit_gemm2(g))
                        )
                # keep PE slightly ahead of ACT but not idle: fill the gap
                pop_fillers(act_t - pe_t)

            # drain remaining fillers in order
            while fillers:
                ready, cost, fn = fillers.pop(0)
                fn()

    nc.compile()
    return nc


def _host_prep(x, Wkqv, bkqv, Wproj, bproj):
    f16 = np.float16
    Wk = Wkqv[:, 0:D]
    Wq = Wkqv[:, D : 2 * D]
    Wv = Wkqv[:, 2 * D : 3 * D]
    bk = bkqv[0:D]
    bq = bkqv[D : 2 * D]
    bv = bkqv[2 * D : 3 * D]
    out_bias = (bproj + bv @ Wproj).astype(np.float32)  # softmax rows sum to 1

    ident = np.eye(P, dtype=f16)
    # btri[k, i] = -30000 where k > i: accumulated into scoresT diag blocks,
    # exp((s - 30000) * 0.125) underflows to exactly 0 in fp16.
    btri = (np.tril(np.full((P, P), -30000.0, np.float32), -1)).astype(f16)

    in_maps = []
    for b in range(B):
        xT = x[b].T.astype(f16)                       # [768, 2048]
        # [isl, pi, dc*512 + c] = xT[128*dc + pi, 512*isl + c]
        xTp = np.ascontiguousarray(
            xT.reshape(6, P, NISL, 512).transpose(2, 1, 0, 3).reshape(NISL, P, 6 * 512)
        )
        for g in range(NG):
            hs = [HPC * g + i for i in range(HPC)]
            wk = [np.asarray(Wk[:, HD * h : HD * h + HD]) for h in hs]
            wq = [np.asarray(Wq[:, HD * h : HD * h + HD]) for h in hs]
            wv = [np.asarray(Wv[:, HD * h : HD * h + HD]) for h in hs]
            # column chunks: ci0 = k01, ci1 = q01, ci2 = k2|q2
            wkq = np.concatenate(
                [wk[0], wk[1], wq[0], wq[1], wk[2], wq[2]], axis=1
            ).astype(np.float32)                       # [768, 384]
            # [ci, pi, dc*128 + c] = wkq[128*dc + pi, 128*ci + c]
            wkqp = np.ascontiguousarray(
                wkq.reshape(6, P, 3, P).transpose(2, 1, 0, 3).reshape(3, P, 6 * P)
            ).astype(f16)
            wv_c = np.concatenate(wv, axis=1).astype(np.float32)   # [768, 192]
            # [pi, dc*192 + c] = wv_c[128*dc + pi, c]
            wvp = np.ascontiguousarray(
                wv_c.reshape(6, P, 192).transpose(1, 0, 2).reshape(P, 6 * 192)
            ).astype(f16)
            wp01 = np.concatenate(
                [Wproj[HD * hs[0] : HD * hs[0] + HD, :],
                 Wproj[HD * hs[1] : HD * hs[1] + HD, :]], axis=0
            ).astype(f16)                              # [128, 768]
            wp2 = np.asarray(
                Wproj[HD * hs[2] : HD * hs[2] + HD, :]
            ).astype(f16)                              # [64, 768]
            bkq = np.zeros((P, 4), np.float32)
            bkq[:, 0] = np.concatenate(
                [bk[HD * hs[0] : HD * hs[0] + HD], bk[HD * hs[1] : HD * hs[1] + HD]]
            )
            bkq[:, 1] = np.concatenate(
                [bq[HD * hs[0] : HD * hs[0] + HD], bq[HD * hs[1] : HD * hs[1] + HD]]
            )
            bkq[0:64, 2] = bk[HD * hs[2] : HD * hs[2] + HD]
            bkq[64:128, 3] = bq[HD * hs[2] : HD * hs[2] + HD]
            in_maps.append(
                dict(xTp=xTp, wkqp=wkqp, wvp=wvp, wp01=wp01, wp2=wp2,
                     bkq=bkq, ident=ident, btri=btri)
            )
    return in_maps, out_bias


def kernel(x, Wkqv, bkqv, Wproj, bproj):
    global _compiled, last_exec_time_ns, last_results
    import concourse.bass_utils as bass_utils

    x = np.asarray(x, np.float32)
    Wkqv = np.asarray(Wkqv, np.float32)
    bkqv = np.asarray(bkqv, np.float32)
    Wproj = np.asarray(Wproj, np.float32)
    bproj = np.asarray(bproj, np.float32)

    if _compiled is None:
        _compiled = _build()
    nc = _compiled

    in_maps, out_bias = _host_prep(x, Wkqv, bkqv, Wproj, bproj)

    trace = os.environ.get("BASS_KERNEL_TRACE", "0") == "1"
    res = bass_utils.run_bass_kernel_spmd(
        nc, in_maps, core_ids=list(range(NCORES)), trace=trace
    )
    last_exec_time_ns = res.exec_time_ns
    last_results = res

    out = np.zeros((B, N, D), np.float32)
    for b in range(B):
        acc = np.zeros((D, N), np.float32)
        for g in range(NG):
            acc += res.results[b * NG + g]["yT"].reshape(D, N).astype(np.float32)
        out[b] = acc.T + out_bias
    return out



# revision 15
# speedup vs baseline: 1.2062x; 1.2062x over previous
"""Causal self-attention (B=2, N=2048, D=768, H=12) on 8 Trainium2 NeuronCores.

Sharding: data-parallel over batch (2) x tensor-parallel over head groups (4),
3 heads per core. Each core computes, for its (batch, head-group):
  GEMM1: kT/qT (transposed) and v (natural) projections from xT,
  scores^T = k @ q^T per head, exp on ScalarE (fp16 out),
  AV with a ones-augmented V giving unnormalized sa + row sums,
  normalize, GEMM2 row-parallel -> yT partial (fp16).
All matmul operands are fp16 (fp32 PSUM accumulate). Host shards inputs, sums
the 4 per-batch partials (the "all-reduce"), and adds the output bias fold
(bproj + bkqv_v @ Wproj - exact because softmax rows sum to 1).

v3 changes vs v2 (trace-driven):
  - exp activations widened to 1024-col PSUM chunks (2-bank sc tiles,
    double-buffered): ~48 fewer ACT instructions, ~10us less ACT busy
  - k/q bias adds moved DVE->ACT (activation Identity with per-partition
    bias AP); they run in phase A where ACT is otherwise idle
  - normalize multiply reads the AV PSUM tile directly (no uav SBUF
    evacuation): ~13us less DVE busy
  - strips emitted head-interleaved; GEMM2 emitted per-oc-chunk
  - filler pop scans past not-yet-ready entries; emits a dummy matmul if
    nothing is ready (a starved PE re-throttles the HAM clock gate)

v4 changes:
  - reciprocal_approx_fast CANNOT read PSUM on hardware (probe: garbage
    results; CoreSim disagrees) - row sum goes through an SBUF copy again
  - ALL matmuls contract over the full 128 partitions: q is stored
    zero-padded per head (qz0=[q0;0], qz1=[0;q1], qz2=[q2;0]), kT1 and
    saT2/wp2 are zero-padded too. Strip/GEMM2 stream cost is unchanged
    (cost = moving cols), but the PE activity monitor appears to weigh
    active rows: K=64-heavy phases ran at K=4/8 clock (1.2GHz) even when
    gap-free, K=128-heavy phases at 2.4GHz. 0*0=0 keeps results exact.
  - GEMM2 readiness keyed off the actual pop time of its head-2 AV (plus
    normalize-chain latency) instead of a static ACT-watermark guess;
    drain phase keeps the PE fed with dummies while normalize chains run.

Self-contained: hardcodes all shapes; no sibling imports.
"""

import os

import numpy as np

B, N, D = 2, 2048, 768
H, HD = 12, 64
HPC = 3           # heads per core
NG = 4            # head groups
NCORES = 8
P = 128
NJ = N // P       # 16 j-chunks (keys) per head
NISL = 4          # 512-query i-slices

_compiled = None  # cached compiled Bass module
last_exec_time_ns = None
last_results = None

N_WARMUP = 13     # 512-wide dummy matmuls bridging boot -> first GEMM1


def _build():
    import concourse.tile as tile
    import concourse.mybir as mybir
    from concourse import bacc

    f32 = mybir.dt.float32
    f16 = mybir.dt.float16
    MULT = mybir.AluOpType.mult
    EXP = mybir.ActivationFunctionType.Exp

    nc = bacc.Bacc(
        "TRN2", target_bir_lowering=False, debug=False, num_devices=NCORES
    )

    # packed DRAM layouts (see _host_prep)
    xT_d = nc.dram_tensor("xTp", [NISL, P, 6 * 512], f16, kind="ExternalInput").ap()
    wkq_d = nc.dram_tensor("wkqp", [3, P, 6 * 128], f16, kind="ExternalInput").ap()
    wv_d = nc.dram_tensor("wvp", [P, 6 * 192], f16, kind="ExternalInput").ap()
    wp01_d = nc.dram_tensor("wp01", [P, D], f16, kind="ExternalInput").ap()
    wp2_d = nc.dram_tensor("wp2", [P, D], f16, kind="ExternalInput").ap()
    bkq_d = nc.dram_tensor("bkq", [P, 4], f32, kind="ExternalInput").ap()
    ident_d = nc.dram_tensor("ident", [P, P], f16, kind="ExternalInput").ap()
    btri_d = nc.dram_tensor("btri", [P, P], f16, kind="ExternalInput").ap()
    yT_d = nc.dram_tensor("yT", [6, P, N], f16, kind="ExternalOutput").ap()

    xT_v = xT_d.rearrange("i p f -> p i f")      # [128, 4, 3072]
    wkq_v = wkq_d.rearrange("c p f -> p c f")    # [128, 3, 768]
    yT_v = yT_d.rearrange("o p f -> p o f")      # [128, 6, 2048]

    with tile.TileContext(nc) as tc:
        import contextlib

        ctx = contextlib.ExitStack()
        with ctx:
            const = ctx.enter_context(tc.tile_pool(name="const", bufs=1))
            big = ctx.enter_context(tc.tile_pool(name="bigbufs", bufs=1))
            work = ctx.enter_context(tc.tile_pool(name="work", bufs=3))
            ypool = ctx.enter_context(tc.tile_pool(name="ypool", bufs=3))
            # PSUM budget (8 banks): sc 2x[128,1024] (4) + gemm 2x[128,512]
            # (2) + av 2x[65,512] (2)
            psum_sc = ctx.enter_context(
                tc.tile_pool(name="psum_sc", bufs=2, space="PSUM")
            )
            psum_gemm = ctx.enter_context(
                tc.tile_pool(name="psum_gemm", bufs=2, space="PSUM")
            )
            psum_av = ctx.enter_context(
                tc.tile_pool(name="psum_av", bufs=2, space="PSUM")
            )

            # ---- SBUF tiles ----
            bkq_t = const.tile([P, 4], f32, name="bkq_t")
            wkq_ts = [const.tile([P, 768], f16, name=f"wkq_t{c}") for c in range(3)]
            xT_ts = [big.tile([P, 6 * 512], f16, name=f"xT_t{i}") for i in range(NISL)]
            ident_t = const.tile([P, P], f16, name="ident_t")
            btri_t = const.tile([P, P], f16, name="btri_t")
            wv_t = const.tile([P, 6 * 192], f16, name="wv_t")
            wp01_t = const.tile([P, D], f16, name="wp01_t")
            wp2_t = const.tile([P, D], f16, name="wp2_t")

            # DMA issue order == priority order (first-needed first)
            nc.sync.dma_start(bkq_t[:], bkq_d)
            nc.sync.dma_start(wkq_ts[0][:], wkq_v[:, 0, :])
            nc.sync.dma_start(xT_ts[0][:], xT_v[:, 0, :])
            nc.sync.dma_start(wkq_ts[1][:], wkq_v[:, 1, :])
            nc.sync.dma_start(xT_ts[1][:], xT_v[:, 1, :])
            nc.sync.dma_start(wkq_ts[2][:], wkq_v[:, 2, :])
            nc.sync.dma_start(ident_t[:], ident_d)
            nc.sync.dma_start(btri_t[:], btri_d)
            nc.sync.dma_start(xT_ts[2][:], xT_v[:, 2, :])
            nc.sync.dma_start(wv_t[:], wv_d)
            nc.sync.dma_start(xT_ts[3][:], xT_v[:, 3, :])
            nc.sync.dma_start(wp01_t[:], wp01_d)
            nc.sync.dma_start(wp2_t[:], wp2_d)

            # PE warmup on a zeroed scratch while the first inputs land
            wscr = const.tile([P, 512], f16, name="wscr")
            nc.vector.memset(wscr[:], 0.0)

            def emit_dummy():
                wps = psum_gemm.tile([P, 512], f32, tag="ps512", name="wps")
                nc.tensor.matmul(
                    wps[:, 0:512], wscr[:, 0:128], wscr[:], start=True, stop=True
                )

            for _ in range(N_WARMUP):
                emit_dummy()

            # k tiles: kT0 = [k0; k1] on 128 partitions, kT1 = [k2; zeros].
            # q tiles zero-padded per head so strip matmuls contract K=128:
            # qz0 = [q0; 0], qz1 = [0; q1], qz2 = [q2; 0]. The zero halves
            # contribute 0 to the scores; full-K keeps the PE clock warm.
            kT0 = big.tile([P, N], f16, name="kT0")
            kT1 = big.tile([P, N], f16, name="kT1")
            qzs = [big.tile([P, N], f16, name=f"qz{h}") for h in range(HPC)]
            q2st = big.tile([P, N], f16, name="q2st")
            nc.vector.memset(kT1[64:128, :], 0.0)
            nc.vector.memset(qzs[0][64:128, :], 0.0)
            nc.vector.memset(qzs[1][0:64, :], 0.0)
            nc.vector.memset(qzs[2][64:128, :], 0.0)
            vaug = big.tile([P, NJ, HPC, 65], f16, name="vaug")
            nc.vector.memset(vaug[:, :, :, 64:65], 1.0)
            # saT: heads 0+1 packed on 128 partitions; head 2 zero-padded
            saT01s = [big.tile([P, 512], f16, name=f"saT01_{i}") for i in range(4)]
            saT2s = [big.tile([P, 512], f16, name=f"saT2_{i}") for i in range(4)]
            for i in range(4):
                nc.vector.memset(saT2s[i][64:128, :], 0.0)

            # ---- GEMM1 k/q: psum tile per (isl, ci) accumulated over dc ----
            # bias add + cast on ACT (idle during phase A)
            def emit_gemm1_kq(isl, ci):
                ps = psum_gemm.tile([P, 512], f32, tag="ps512", name="ps_kq")
                for dc in range(6):
                    nc.tensor.matmul(
                        ps[:, 0:512],
                        wkq_ts[ci][:, 128 * dc : 128 * dc + 128],
                        xT_ts[isl][:, 512 * dc : 512 * dc + 512],
                        start=(dc == 0),
                        stop=(dc == 5),
                    )
                sl = slice(512 * isl, 512 * isl + 512)
                if ci == 0:
                    nc.scalar.add(kT0[:, sl], ps[:, 0:512], bkq_t[:, 0:1])
                elif ci == 1:
                    nc.scalar.add(
                        qzs[0][0:64, sl], ps[0:64, 0:512], bkq_t[0:64, 1:2]
                    )
                    nc.scalar.add(
                        qzs[1][64:128, sl], ps[64:128, 0:512], bkq_t[64:128, 1:2]
                    )
                else:
                    nc.scalar.add(
                        kT1[0:64, sl], ps[0:64, 0:512], bkq_t[0:64, 2:3]
                    )
                    nc.scalar.add(
                        q2st[64:128, sl], ps[64:128, 0:512], bkq_t[64:128, 3:4]
                    )
                    nc.sync.dma_start(qzs[2][0:64, sl], q2st[64:128, sl])

            # ---- GEMM1 v: one psum tile per 128-query chunk ----
            def emit_gemm1_v(ic):
                ps = psum_gemm.tile([P, 512], f32, tag="ps512", name="ps_v")
                isl, k = divmod(ic, 4)
                for dc in range(6):
                    nc.tensor.matmul(
                        ps[:, 0:192],
                        xT_ts[isl][:, 512 * dc + 128 * k : 512 * dc + 128 * k + 128],
                        wv_t[:, 192 * dc : 192 * dc + 192],
                        start=(dc == 0),
                        stop=(dc == 5),
                    )
                nc.vector.tensor_copy(
                    out=vaug[:, ic, :, 0:64],
                    in_=ps[:, 0:192].rearrange("p (h d) -> p h d", h=HPC),
                )

            # ---- strips: scoresT + causal mask + exp (1024-wide chunks) ----
            all_strips = [[None] * NJ for _ in range(HPC)]

            def emit_strip(h, jc):
                kTc = kT0 if h < 2 else kT1
                qTc = qzs[h]
                i0 = 128 * jc
                W = N - i0
                strip = work.tile(
                    [P, W], f16, tag=f"expT{jc}", bufs=3, name=f"expT{jc}"
                )
                for c0 in range(0, W, 1024):
                    cw = min(1024, W - c0)
                    ps = psum_sc.tile([P, 1024], f32, tag="sc", name="ps_s")
                    for s0 in range(c0, c0 + cw, 512):
                        sw = min(512, W - s0)
                        chained = s0 == 0
                        nc.tensor.matmul(
                            ps[:, s0 - c0 : s0 - c0 + sw],
                            kTc[:, i0 : i0 + 128],
                            qTc[:, i0 + s0 : i0 + s0 + sw],
                            start=True,
                            stop=(not chained),
                        )
                        if chained:
                            # causal mask: accumulate -30000 above the diagonal
                            nc.tensor.matmul(
                                ps[:, 0:128], ident_t[:], btri_t[:],
                                start=False, stop=True,
                            )
                    nc.scalar.activation(
                        strip[:, c0 : c0 + cw], ps[:, 0:cw], EXP, scale=0.125
                    )
                all_strips[h][jc] = strip

            # ---- AV + normalize (normalize reads the AV PSUM directly) ----
            def emit_av(h, iseg):
                strips = all_strips[h]
                ps2 = psum_av.tile([65, 512], f32, tag="av", name="ps2")
                jmax = 4 * iseg + 3
                for jc in range(jmax + 1):
                    off = 512 * iseg - 128 * jc
                    lo = max(0, off)
                    w = 512 - (lo - off)
                    nc.tensor.matmul(
                        ps2[0:65, 512 - w : 512],
                        vaug[:, jc, h, :],
                        strips[jc][:, lo : lo + w],
                        start=(jc == 0),
                        stop=(jc == jmax),
                    )
                # row sum must bounce through SBUF: reciprocal_approx_fast
                # reads garbage from PSUM on hardware. The multiply below can
                # read PSUM directly. GpSimd runs ONLY partition_broadcast
                # (mixing op types forces library reloads).
                srow = work.tile([1, 512], f32, tag="srow", bufs=2, name="srow")
                nc.vector.tensor_copy(out=srow[:], in_=ps2[64:65, :])
                rrow = work.tile([1, 512], f32, tag="rrow", bufs=2, name="rrow")
                nc.vector.reciprocal_approx_fast(out=rrow[:], in_=srow[:])
                rbc = work.tile([64, 512], f32, tag="rbc", bufs=2, name="rbc")
                nc.gpsimd.partition_broadcast(rbc[:], rrow[:])
                if h == 0:
                    nc.vector.tensor_tensor(
                        saT01s[iseg][0:64, :], ps2[0:64, :], rbc[:], MULT
                    )
                elif h == 1:
                    st1 = work.tile([64, 512], f16, tag="st1", bufs=2, name="st1")
                    nc.vector.tensor_tensor(st1[:], ps2[0:64, :], rbc[:], MULT)
                    nc.sync.dma_start(saT01s[iseg][64:128, :], st1[:])
                else:
                    nc.vector.tensor_tensor(
                        saT2s[iseg][0:64, :], ps2[0:64, :], rbc[:], MULT
                    )

            # ---- GEMM2: heads 0+1 contract-128, head 2 contract-64 ----
            def emit_gemm2_oc(isl, oc):
                ps = psum_gemm.tile([P, 512], f32, tag="ps512", name="ps_y")
                nc.tensor.matmul(
                    ps[:, 0:512],
                    wp01_t[:, 128 * oc : 128 * oc + 128],
                    saT01s[isl][:],
                    start=True,
                    stop=False,
                )
                nc.tensor.matmul(
                    ps[:, 0:512],
                    wp2_t[:, 128 * oc : 128 * oc + 128],
                    saT2s[isl][:, :],
                    start=False,
                    stop=True,
                )
                yst = ypool.tile([P, 512], f16, tag="yst", name="yst")
                nc.vector.tensor_copy(out=yst[:], in_=ps[:, 0:512])
                nc.sync.dma_start(
                    yT_v[:, oc, 512 * isl : 512 * isl + 512], yst[:]
                )

            # ---- emission schedule ----
            # Phase A: all k/q projections. ci2 must precede the first h2
            # strip in the PE FIFO (in-order queue: a consumer ahead of its
            # producer deadlocks), so it is not eligible as pacing filler.
            for isl in range(NISL):
                emit_gemm1_kq(isl, 0)
                emit_gemm1_kq(isl, 1)
            for isl in range(NISL):
                emit_gemm1_kq(isl, 2)

            # Phase B: strips paced against ACT, with independent PE work
            # (v projections, AV groups, GEMM2 chunks) as filler.
            # Costs in ns for the pacing model (warm clock).
            def strip_pe_cost(W):
                return W / 2.4 + 110 * ((W + 511) // 512) + 160

            def strip_act_cost(W):
                return 0.85 * W + 300 * ((W + 1023) // 1024)

            # fillers: mutable [ready_gate, pe_cost, emit_fn] entries. The
            # gate compares against pe_t (emitted-PE-work watermark).
            fillers = []
            for ic in range(16):
                fillers.append([0.0, 580.0, lambda ic=ic: emit_gemm1_v(ic)])

            pe_t = 0.0    # PE-busy time emitted so far (phase B origin)
            act_t = 0.0   # ACT-busy time emitted so far
            SLACK = 3000.0
            NORM_DELAY = 4500.0   # AV pop -> saT ready (AV + recip+bcast+mult)
            n_dummy = 0

            def emit_av_tracked(h, g, gemm2_entries):
                emit_av(h, g)
                if h == HPC - 1:
                    # GEMM2 for group g becomes available once the normalize
                    # chains (running on DVE/GpSimd) have drained
                    for e in gemm2_entries:
                        e[0] = pe_t + NORM_DELAY

            def pop_fillers(budget, allow_dummy=True):
                # Pop ready fillers (scanning past not-yet-ready ones; safe:
                # GEMM2 gates open only after its AV deps were popped).
                # If nothing is ready and a real deficit remains, emit a
                # dummy matmul: a starved PE re-throttles the HAM clock.
                nonlocal pe_t, n_dummy
                spent = 0.0
                while fillers and spent < budget:
                    for i, e in enumerate(fillers):
                        if e[0] <= pe_t:
                            fillers.pop(i)
                            e[2]()
                            pe_t += e[1]
                            spent += e[1]
                            break
                    else:
                        if allow_dummy and budget - spent > 600.0 and n_dummy < 90:
                            emit_dummy()
                            n_dummy += 1
                            pe_t += 215.0
                            spent += 215.0
                        else:
                            break
                return spent

            strip_order = []
            for g in range(4):
                for jc in range(4 * g, 4 * g + 4):
                    for h in range(HPC):
                        strip_order.append((h, jc, g))

            for h, jc, g in strip_order:
                W = N - 128 * jc
                emit_strip(h, jc)
                pe_t += strip_pe_cost(W)
                act_t += strip_act_cost(W)
                if jc == 4 * g + 3 and h == HPC - 1:
                    # all strips for group g emitted: AV groups become
                    # available once ACT has drained through them
                    av_cols = 512 * (4 * g + 1) + 768
                    gemm2_entries = [
                        [float("inf"), 620.0,
                         lambda g=g, oc=oc: emit_gemm2_oc(g, oc)]
                        for oc in range(6)
                    ]
                    for hh in range(HPC):
                        fillers.append(
                            [act_t + SLACK, av_cols / 2.4 + 150,
                             lambda hh=hh, g=g, ge=gemm2_entries:
                                 emit_av_tracked(hh, g, ge)]
                        )
                    fillers.extend(gemm2_entries)
                # keep PE slightly ahead of ACT but not idle: fill the gap
                pop_fillers(act_t - pe_t)

            # drain: keep popping; feed dummies while gates (normalize
            # chains) are still closed, then force-pop in order
            while fillers:
                if pop_fillers(1e9, allow_dummy=False) == 0.0:
                    if n_dummy < 90:
                        emit_dummy()
                        n_dummy += 1
                        pe_t += 215.0
                    else:
                        e = fillers.pop(0)
                        e[2]()
                        pe_t += e[1]

    nc.compile()
    return nc


def _host_prep(x, Wkqv, bkqv, Wproj, bproj):
    f16 = np.float16
    Wk = Wkqv[:, 0:D]
    Wq = Wkqv[:, D : 2 * D]
    Wv = Wkqv[:, 2 * D : 3 * D]
    bk = bkqv[0:D]
    bq = bkqv[D : 2 * D]
    bv = bkqv[2 * D : 3 * D]
    out_bias = (bproj + bv @ Wproj).astype(np.float32)  # softmax rows sum to 1

    ident = np.eye(P, dtype=f16)
    # btri[k, i] = -30000 where k > i: accumulated into scoresT diag blocks,
    # exp((s - 30000) * 0.125) underflows to exactly 0 in fp16.
    btri = (np.tril(np.full((P, P), -30000.0, np.float32), -1)).astype(f16)

    in_maps = []
    for b in range(B):
        xT = x[b].T.astype(f16)                       # [768, 2048]
        # [isl, pi, dc*512 + c] = xT[128*dc + pi, 512*isl + c]
        xTp = np.ascontiguousarray(
            xT.reshape(6, P, NISL, 512).transpose(2, 1, 0, 3).reshape(NISL, P, 6 * 512)
        )
        for g in range(NG):
            hs = [HPC * g + i for i in range(HPC)]
            wk = [np.asarray(Wk[:, HD * h : HD * h + HD]) for h in hs]
            wq = [np.asarray(Wq[:, HD * h : HD * h + HD]) for h in hs]
            wv = [np.asarray(Wv[:, HD * h : HD * h + HD]) for h in hs]
            # column chunks: ci0 = k01, ci1 = q01, ci2 = k2|q2
            wkq = np.concatenate(
                [wk[0], wk[1], wq[0], wq[1], wk[2], wq[2]], axis=1
            ).astype(np.float32)                       # [768, 384]
            # [ci, pi, dc*128 + c] = wkq[128*dc + pi, 128*ci + c]
            wkqp = np.ascontiguousarray(
                wkq.reshape(6, P, 3, P).transpose(2, 1, 0, 3).reshape(3, P, 6 * P)
            ).astype(f16)
            wv_c = np.concatenate(wv, axis=1).astype(np.float32)   # [768, 192]
            # [pi, dc*192 + c] = wv_c[128*dc + pi, c]
            wvp = np.ascontiguousarray(
                wv_c.reshape(6, P, 192).transpose(1, 0, 2).reshape(P, 6 * 192)
            ).astype(f16)
            wp01 = np.concatenate(
                [Wproj[HD * hs[0] : HD * hs[0] + HD, :],
                 Wproj[HD * hs[1] : HD * hs[1] + HD, :]], axis=0
            ).astype(f16)                              # [128, 768]
            wp2 = np.zeros((P, D), f16)                # [128, 768], rows 64+ zero
            wp2[0:64, :] = Wproj[HD * hs[2] : HD * hs[2] + HD, :].astype(f16)
            bkq = np.zeros((P, 4), np.float32)
            bkq[:, 0] = np.concatenate(
                [bk[HD * hs[0] : HD * hs[0] + HD], bk[HD * hs[1] : HD * hs[1] + HD]]
            )
            bkq[:, 1] = np.concatenate(
                [bq[HD * hs[0] : HD * hs[0] + HD], bq[HD * hs[1] : HD * hs[1] + HD]]
            )
            bkq[0:64, 2] = bk[HD * hs[2] : HD * hs[2] + HD]
            bkq[64:128, 3] = bq[HD * hs[2] : HD * hs[2] + HD]
            in_maps.append(
                dict(xTp=xTp, wkqp=wkqp, wvp=wvp, wp01=wp01, wp2=wp2,
                     bkq=bkq, ident=ident, btri=btri)
            )
    return in_maps, out_bias


def kernel(x, Wkqv, bkqv, Wproj, bproj):
    global _compiled, last_exec_time_ns, last_results
    import concourse.bass_utils as bass_utils

    x = np.asarray(x, np.float32)
    Wkqv = np.asarray(Wkqv, np.float32)
    bkqv = np.asarray(bkqv, np.float32)
    Wproj = np.asarray(Wproj, np.float32)
    bproj = np.asarray(bproj, np.float32)

    if _compiled is None:
        _compiled = _build()
    nc = _compiled

    in_maps, out_bias = _host_prep(x, Wkqv, bkqv, Wproj, bproj)

    trace = os.environ.get("BASS_KERNEL_TRACE", "0") == "1"
    res = bass_utils.run_bass_kernel_spmd(
        nc, in_maps, core_ids=list(range(NCORES)), trace=trace
    )
    last_exec_time_ns = res.exec_time_ns
    last_results = res

    out = np.zeros((B, N, D), np.float32)
    for b in range(B):
        acc = np.zeros((D, N), np.float32)
        for g in range(NG):
            acc += res.results[b * NG + g]["yT"].reshape(D, N).astype(np.float32)
        out[b] = acc.T + out_bias
    return out


# revision 20
# speedup vs baseline: 1.2750x; 1.0570x over previous
"""Causal self-attention (B=2, N=2048, D=768, H=12) on 8 Trainium2 NeuronCores.

Sharding: data-parallel over batch (2) x tensor-parallel over head groups (4),
3 heads per core. Each core computes, for its (batch, head-group):
  GEMM1: kT/qT (transposed) and v (natural) projections from xT,
  scores^T = k @ q^T per head, exp on ScalarE (fp16 out),
  AV with a ones-augmented V giving unnormalized sa + row sums,
  normalize, GEMM2 row-parallel -> yT partial (fp16).
All matmul operands are fp16 (fp32 PSUM accumulate). Host shards inputs, sums
the 4 per-batch partials (the "all-reduce"), and adds the output bias fold
(bproj + bkqv_v @ Wproj - exact because softmax rows sum to 1).

v3 changes vs v2 (trace-driven):
  - exp activations widened to 1024-col PSUM chunks (2-bank sc tiles,
    double-buffered): ~48 fewer ACT instructions, ~10us less ACT busy
  - k/q bias adds moved DVE->ACT (activation Identity with per-partition
    bias AP); they run in phase A where ACT is otherwise idle
  - normalize multiply reads the AV PSUM tile directly (no uav SBUF
    evacuation): ~13us less DVE busy
  - strips emitted head-interleaved; GEMM2 emitted per-oc-chunk
  - filler pop scans past not-yet-ready entries; emits a dummy matmul if
    nothing is ready (a starved PE re-throttles the HAM clock gate)

v4 changes:
  - reciprocal_approx_fast CANNOT read PSUM on hardware (probe: garbage
    results; CoreSim disagrees) - row sum goes through an SBUF copy again
  - ALL matmuls contract over the full 128 partitions: q is stored
    zero-padded per head (qz0=[q0;0], qz1=[0;q1], qz2=[q2;0]), kT1 and
    saT2/wp2 are zero-padded too. Strip/GEMM2 stream cost is unchanged
    (cost = moving cols), but the PE activity monitor appears to weigh
    active rows: K=64-heavy phases ran at K=4/8 clock (1.2GHz) even when
    gap-free, K=128-heavy phases at 2.4GHz. 0*0=0 keeps results exact.
  - GEMM2 readiness keyed off the actual pop time of its head-2 AV (plus
    normalize-chain latency) instead of a static ACT-watermark guess;
    drain phase keeps the PE fed with dummies while normalize chains run.

Self-contained: hardcodes all shapes; no sibling imports.
"""

import os

import numpy as np

B, N, D = 2, 2048, 768
H, HD = 12, 64
HPC = 3           # heads per core
NG = 4            # head groups
NCORES = 8
P = 128
NJ = N // P       # 16 j-chunks (keys) per head
NISL = 4          # 512-query i-slices

_compiled = None  # cached compiled Bass module
last_exec_time_ns = None
last_results = None

N_WARMUP = 13     # 512-wide dummy matmuls bridging boot -> first GEMM1


def _build():
    import concourse.tile as tile
    import concourse.mybir as mybir
    from concourse import bacc

    f32 = mybir.dt.float32
    f16 = mybir.dt.float16
    MULT = mybir.AluOpType.mult
    EXP = mybir.ActivationFunctionType.Exp

    nc = bacc.Bacc(
        "TRN2", target_bir_lowering=False, debug=False, num_devices=NCORES
    )

    # packed DRAM layouts (see _host_prep)
    xT_d = nc.dram_tensor("xTp", [NISL, P, 6 * 512], f16, kind="ExternalInput").ap()
    wkq_d = nc.dram_tensor("wkqp", [3, P, 6 * 128], f16, kind="ExternalInput").ap()
    wv_d = nc.dram_tensor("wvp", [P, 6 * 192], f16, kind="ExternalInput").ap()
    wp01_d = nc.dram_tensor("wp01", [P, D], f16, kind="ExternalInput").ap()
    wp2_d = nc.dram_tensor("wp2", [P, D], f16, kind="ExternalInput").ap()
    bkq_d = nc.dram_tensor("bkq", [P, 4], f32, kind="ExternalInput").ap()
    ident_d = nc.dram_tensor("ident", [P, P], f16, kind="ExternalInput").ap()
    btri_d = nc.dram_tensor("btri", [P, P], f16, kind="ExternalInput").ap()
    yT_d = nc.dram_tensor("yT", [6, P, N], f16, kind="ExternalOutput").ap()

    xT_v = xT_d.rearrange("i p f -> p i f")      # [128, 4, 3072]
    wkq_v = wkq_d.rearrange("c p f -> p c f")    # [128, 3, 768]
    yT_v = yT_d.rearrange("o p f -> p o f")      # [128, 6, 2048]

    with tile.TileContext(nc) as tc:
        import contextlib

        ctx = contextlib.ExitStack()
        with ctx:
            const = ctx.enter_context(tc.tile_pool(name="const", bufs=1))
            big = ctx.enter_context(tc.tile_pool(name="bigbufs", bufs=1))
            work = ctx.enter_context(tc.tile_pool(name="work", bufs=3))
            ypool = ctx.enter_context(tc.tile_pool(name="ypool", bufs=3))
            # PSUM budget (8 banks): sc 2x[128,1024] (4) + gemm 2x[128,512]
            # (2) + av 2x[65,512] (2)
            psum_sc = ctx.enter_context(
                tc.tile_pool(name="psum_sc", bufs=2, space="PSUM")
            )
            psum_gemm = ctx.enter_context(
                tc.tile_pool(name="psum_gemm", bufs=2, space="PSUM")
            )
            psum_av = ctx.enter_context(
                tc.tile_pool(name="psum_av", bufs=2, space="PSUM")
            )

            # ---- SBUF tiles ----
            bkq_t = const.tile([P, 4], f32, name="bkq_t")
            wkq_ts = [const.tile([P, 768], f16, name=f"wkq_t{c}") for c in range(3)]
            xT_ts = [big.tile([P, 6 * 512], f16, name=f"xT_t{i}") for i in range(NISL)]
            ident_t = const.tile([P, P], f16, name="ident_t")
            btri_t = const.tile([P, P], f16, name="btri_t")
            wv_t = const.tile([P, 6 * 192], f16, name="wv_t")
            wp01_t = const.tile([P, D], f16, name="wp01_t")
            wp2_t = const.tile([P, D], f16, name="wp2_t")

            # DMA issue order == priority order (first-needed first)
            nc.sync.dma_start(bkq_t[:], bkq_d)
            nc.sync.dma_start(wkq_ts[0][:], wkq_v[:, 0, :])
            nc.sync.dma_start(xT_ts[0][:], xT_v[:, 0, :])
            nc.sync.dma_start(wkq_ts[1][:], wkq_v[:, 1, :])
            nc.sync.dma_start(xT_ts[1][:], xT_v[:, 1, :])
            nc.sync.dma_start(wkq_ts[2][:], wkq_v[:, 2, :])
            nc.sync.dma_start(ident_t[:], ident_d)
            nc.sync.dma_start(btri_t[:], btri_d)
            nc.sync.dma_start(xT_ts[2][:], xT_v[:, 2, :])
            nc.sync.dma_start(wv_t[:], wv_d)
            nc.sync.dma_start(xT_ts[3][:], xT_v[:, 3, :])
            nc.sync.dma_start(wp01_t[:], wp01_d)
            nc.sync.dma_start(wp2_t[:], wp2_d)

            # PE warmup on a zeroed scratch while the first inputs land
            wscr = const.tile([P, 512], f16, name="wscr")
            nc.vector.memset(wscr[:], 0.0)

            def emit_dummy():
                wps = psum_gemm.tile([P, 512], f32, tag="ps512", name="wps")
                nc.tensor.matmul(
                    wps[:, 0:512], wscr[:, 0:128], wscr[:], start=True, stop=True
                )

            for _ in range(N_WARMUP):
                emit_dummy()

            # k tiles: kT0 = [k0; k1] on 128 partitions, kT1 = [k2; zeros].
            # q tiles zero-padded per head so strip matmuls contract K=128:
            # qz0 = [q0; 0], qz1 = [0; q1], qz2 = [q2; 0]. The zero halves
            # contribute 0 to the scores; full-K keeps the PE clock warm.
            kT0 = big.tile([P, N], f16, name="kT0")
            kT1 = big.tile([P, N], f16, name="kT1")
            qzs = [big.tile([P, N], f16, name=f"qz{h}") for h in range(HPC)]
            q2st = big.tile([P, N], f16, name="q2st")
            nc.vector.memset(kT1[64:128, :], 0.0)
            nc.vector.memset(qzs[0][64:128, :], 0.0)
            nc.vector.memset(qzs[1][0:64, :], 0.0)
            nc.vector.memset(qzs[2][64:128, :], 0.0)
            vaug = big.tile([P, NJ, HPC, 65], f16, name="vaug")
            nc.vector.memset(vaug[:, :, :, 64:65], 1.0)
            # saT: heads 0+1 packed on 128 partitions; head 2 zero-padded
            saT01s = [big.tile([P, 512], f16, name=f"saT01_{i}") for i in range(4)]
            saT2s = [big.tile([P, 512], f16, name=f"saT2_{i}") for i in range(4)]
            for i in range(4):
                nc.vector.memset(saT2s[i][64:128, :], 0.0)

            # ---- GEMM1 k/q: psum tile per (isl, ci) accumulated over dc ----
            # bias add + cast on ACT (idle during phase A)
            def emit_gemm1_kq(isl, ci):
                ps = psum_gemm.tile([P, 512], f32, tag="ps512", name="ps_kq")
                for dc in range(6):
                    nc.tensor.matmul(
                        ps[:, 0:512],
                        wkq_ts[ci][:, 128 * dc : 128 * dc + 128],
                        xT_ts[isl][:, 512 * dc : 512 * dc + 512],
                        start=(dc == 0),
                        stop=(dc == 5),
                    )
                sl = slice(512 * isl, 512 * isl + 512)
                if ci == 0:
                    nc.scalar.add(kT0[:, sl], ps[:, 0:512], bkq_t[:, 0:1])
                elif ci == 1:
                    nc.scalar.add(
                        qzs[0][0:64, sl], ps[0:64, 0:512], bkq_t[0:64, 1:2]
                    )
                    nc.scalar.add(
                        qzs[1][64:128, sl], ps[64:128, 0:512], bkq_t[64:128, 1:2]
                    )
                else:
                    nc.scalar.add(
                        kT1[0:64, sl], ps[0:64, 0:512], bkq_t[0:64, 2:3]
                    )
                    nc.scalar.add(
                        q2st[64:128, sl], ps[64:128, 0:512], bkq_t[64:128, 3:4]
                    )
                    nc.sync.dma_start(qzs[2][0:64, sl], q2st[64:128, sl])

            # ---- GEMM1 v: one psum tile per 128-query chunk ----
            def emit_gemm1_v(ic):
                ps = psum_gemm.tile([P, 512], f32, tag="ps512", name="ps_v")
                isl, k = divmod(ic, 4)
                for dc in range(6):
                    nc.tensor.matmul(
                        ps[:, 0:192],
                        xT_ts[isl][:, 512 * dc + 128 * k : 512 * dc + 128 * k + 128],
                        wv_t[:, 192 * dc : 192 * dc + 192],
                        start=(dc == 0),
                        stop=(dc == 5),
                    )
                nc.vector.tensor_copy(
                    out=vaug[:, ic, :, 0:64],
                    in_=ps[:, 0:192].rearrange("p (h d) -> p h d", h=HPC),
                )

            # ---- strips: scoresT + causal mask + exp (1024-wide chunks) ----
            all_strips = [[None] * NJ for _ in range(HPC)]

            def emit_strip(h, jc):
                kTc = kT0 if h < 2 else kT1
                qTc = qzs[h]
                i0 = 128 * jc
                W = N - i0
                strip = work.tile(
                    [P, W], f16, tag=f"expT{jc}", bufs=3, name=f"expT{jc}"
                )
                for c0 in range(0, W, 1024):
                    cw = min(1024, W - c0)
                    ps = psum_sc.tile([P, 1024], f32, tag="sc", name="ps_s")
                    for s0 in range(c0, c0 + cw, 512):
                        sw = min(512, W - s0)
                        chained = s0 == 0
                        nc.tensor.matmul(
                            ps[:, s0 - c0 : s0 - c0 + sw],
                            kTc[:, i0 : i0 + 128],
                            qTc[:, i0 + s0 : i0 + s0 + sw],
                            start=True,
                            stop=(not chained),
                        )
                        if chained:
                            # causal mask: accumulate -30000 above the diagonal
                            nc.tensor.matmul(
                                ps[:, 0:128], ident_t[:], btri_t[:],
                                start=False, stop=True,
                            )
                    nc.scalar.activation(
                        strip[:, c0 : c0 + cw], ps[:, 0:cw], EXP, scale=0.125
                    )
                all_strips[h][jc] = strip

            # ---- AV + normalize (normalize reads the AV PSUM directly) ----
            def emit_av(h, iseg):
                strips = all_strips[h]
                ps2 = psum_av.tile([65, 512], f32, tag="av", name="ps2")
                jmax = 4 * iseg + 3
                for jc in range(jmax + 1):
                    off = 512 * iseg - 128 * jc
                    lo = max(0, off)
                    w = 512 - (lo - off)
                    nc.tensor.matmul(
                        ps2[0:65, 512 - w : 512],
                        vaug[:, jc, h, :],
                        strips[jc][:, lo : lo + w],
                        start=(jc == 0),
                        stop=(jc == jmax),
                    )
                # row sum must bounce through SBUF: reciprocal_approx_fast
                # reads garbage from PSUM on hardware. The multiply below can
                # read PSUM directly. GpSimd runs ONLY partition_broadcast
                # (mixing op types forces library reloads).
                srow = work.tile([1, 512], f32, tag="srow", bufs=2, name="srow")
                nc.vector.tensor_copy(out=srow[:], in_=ps2[64:65, :])
                rrow = work.tile([1, 512], f32, tag="rrow", bufs=2, name="rrow")
                nc.vector.reciprocal_approx_fast(out=rrow[:], in_=srow[:])
                rbc = work.tile([64, 512], f32, tag="rbc", bufs=2, name="rbc")
                nc.gpsimd.partition_broadcast(rbc[:], rrow[:])
                if h == 0:
                    nc.vector.tensor_tensor(
                        saT01s[iseg][0:64, :], ps2[0:64, :], rbc[:], MULT
                    )
                elif h == 1:
                    st1 = work.tile([64, 512], f16, tag="st1", bufs=2, name="st1")
                    nc.vector.tensor_tensor(st1[:], ps2[0:64, :], rbc[:], MULT)
                    nc.sync.dma_start(saT01s[iseg][64:128, :], st1[:])
                else:
                    nc.vector.tensor_tensor(
                        saT2s[iseg][0:64, :], ps2[0:64, :], rbc[:], MULT
                    )

            # ---- GEMM2: heads 0+1 contract-128, head 2 contract-64 ----
            def emit_gemm2_oc(isl, oc):
                ps = psum_gemm.tile([P, 512], f32, tag="ps512", name="ps_y")
                nc.tensor.matmul(
                    ps[:, 0:512],
                    wp01_t[:, 128 * oc : 128 * oc + 128],
                    saT01s[isl][:],
                    start=True,
                    stop=False,
                )
                nc.tensor.matmul(
                    ps[:, 0:512],
                    wp2_t[:, 128 * oc : 128 * oc + 128],
                    saT2s[isl][:, :],
                    start=False,
                    stop=True,
                )
                yst = ypool.tile([P, 512], f16, tag="yst", name="yst")
                nc.vector.tensor_copy(out=yst[:], in_=ps[:, 0:512])
                nc.sync.dma_start(
                    yT_v[:, oc, 512 * isl : 512 * isl + 512], yst[:]
                )

            # ---- emission schedule ----
            # Phase A: all k/q projections. ci2 must precede the first h2
            # strip in the PE FIFO (in-order queue: a consumer ahead of its
            # producer deadlocks), so it is not eligible as pacing filler.
            for isl in range(NISL):
                emit_gemm1_kq(isl, 0)
                emit_gemm1_kq(isl, 1)
            for isl in range(NISL):
                emit_gemm1_kq(isl, 2)

            # Phase B: strips paced against ACT, with independent PE work
            # (v projections, AV groups, GEMM2 chunks) as filler.
            # Costs in ns for the pacing model (warm clock).
            def strip_pe_cost(W):
                return W / 2.4 + 110 * ((W + 511) // 512) + 160

            def strip_act_cost(W):
                # calibrated: measured exp busy = 0.833ns/col + ~210ns/chunk
                return 0.833 * W + 210 * ((W + 1023) // 1024)

            # fillers: mutable [ready_gate, pe_cost, emit_fn] entries. The
            # gate compares against pe_t (emitted-PE-work watermark).
            fillers = []
            for ic in range(16):
                fillers.append([0.0, 580.0, lambda ic=ic: emit_gemm1_v(ic)])

            pe_t = 0.0    # PE-busy time emitted so far (phase B origin)
            act_t = 0.0   # ACT-busy time emitted so far
            SLACK = 3000.0
            NORM_DELAY = 3000.0   # AV drain -> saT ready (recip+bcast+mult)
            n_dummy = 0

            def emit_av_tracked(h, g, cost, gemm2_entries):
                emit_av(h, g)
                if h == HPC - 1:
                    # GEMM2 for group g becomes available once this AV has
                    # drained (cost) and the normalize chains (DVE/GpSimd)
                    # have run
                    for e in gemm2_entries:
                        e[0] = pe_t + cost + NORM_DELAY

            def pop_fillers(budget, allow_dummy=True):
                # Pop ready fillers (scanning past not-yet-ready ones; safe:
                # GEMM2 gates open only after its AV deps were popped).
                # If nothing is ready and a real deficit remains, emit a
                # dummy matmul: a starved PE re-throttles the HAM clock.
                nonlocal pe_t, n_dummy
                spent = 0.0
                while fillers and spent < budget:
                    for i, e in enumerate(fillers):
                        if e[0] <= pe_t:
                            fillers.pop(i)
                            e[2]()
                            pe_t += e[1]
                            spent += e[1]
                            break
                    else:
                        if allow_dummy and budget - spent > 400.0 and n_dummy < 150:
                            emit_dummy()
                            n_dummy += 1
                            pe_t += 215.0
                            spent += 215.0
                        else:
                            break
                return spent

            # h-major within each group: AV(h, g) readiness staggers per
            # head, keeping the filler supply smooth
            strip_order = []
            for g in range(4):
                for h in range(HPC):
                    for jc in range(4 * g, 4 * g + 4):
                        strip_order.append((h, jc, g))

            gemm2_entries = {}
            for h, jc, g in strip_order:
                W = N - 128 * jc
                emit_strip(h, jc)
                pe_t += strip_pe_cost(W)
                act_t += strip_act_cost(W)
                if jc == 4 * g + 3:
                    # head h's strips for group g all emitted: its AV becomes
                    # available once ACT has drained through them
                    if h == 0:
                        gemm2_entries[g] = [
                            [float("inf"), 620.0,
                             lambda g=g, oc=oc: emit_gemm2_oc(g, oc)]
                            for oc in range(6)
                        ]
                    av_cols = 512 * (4 * g + 1) + 768
                    av_cost = av_cols / 2.4 + 150
                    fillers.append(
                        [act_t + SLACK, av_cost,
                         lambda h=h, g=g, c=av_cost, ge=gemm2_entries[g]:
                             emit_av_tracked(h, g, c, ge)]
                    )
                    if h == HPC - 1:
                        fillers.extend(gemm2_entries[g])
                # keep PE slightly ahead of ACT but not idle: fill the gap
                pop_fillers(act_t - pe_t)

            # drain: keep popping; feed dummies while gates (normalize
            # chains) are still closed, then force-pop in order
            while fillers:
                if pop_fillers(1e9, allow_dummy=False) == 0.0:
                    if n_dummy < 150:
                        emit_dummy()
                        n_dummy += 1
                        pe_t += 215.0
                    else:
                        e = fillers.pop(0)
                        e[2]()
                        pe_t += e[1]

    nc.compile()
    return nc


def _host_prep(x, Wkqv, bkqv, Wproj, bproj):
    f16 = np.float16
    Wk = Wkqv[:, 0:D]
    Wq = Wkqv[:, D : 2 * D]
    Wv = Wkqv[:, 2 * D : 3 * D]
    bk = bkqv[0:D]
    bq = bkqv[D : 2 * D]
    bv = bkqv[2 * D : 3 * D]
    out_bias = (bproj + bv @ Wproj).astype(np.float32)  # softmax rows sum to 1

    ident = np.eye(P, dtype=f16)
    # btri[k, i] = -30000 where k > i: accumulated into scoresT diag blocks,
    # exp((s - 30000) * 0.125) underflows to exactly 0 in fp16.
    btri = (np.tril(np.full((P, P), -30000.0, np.float32), -1)).astype(f16)

    in_maps = []
    for b in range(B):
        xT = x[b].T.astype(f16)                       # [768, 2048]
        # [isl, pi, dc*512 + c] = xT[128*dc + pi, 512*isl + c]
        xTp = np.ascontiguousarray(
            xT.reshape(6, P, NISL, 512).transpose(2, 1, 0, 3).reshape(NISL, P, 6 * 512)
        )
        for g in range(NG):
            hs = [HPC * g + i for i in range(HPC)]
            wk = [np.asarray(Wk[:, HD * h : HD * h + HD]) for h in hs]
            wq = [np.asarray(Wq[:, HD * h : HD * h + HD]) for h in hs]
            wv = [np.asarray(Wv[:, HD * h : HD * h + HD]) for h in hs]
            # column chunks: ci0 = k01, ci1 = q01, ci2 = k2|q2
            wkq = np.concatenate(
                [wk[0], wk[1], wq[0], wq[1], wk[2], wq[2]], axis=1
            ).astype(np.float32)                       # [768, 384]
            # [ci, pi, dc*128 + c] = wkq[128*dc + pi, 128*ci + c]
            wkqp = np.ascontiguousarray(
                wkq.reshape(6, P, 3, P).transpose(2, 1, 0, 3).reshape(3, P, 6 * P)
            ).astype(f16)
            wv_c = np.concatenate(wv, axis=1).astype(np.float32)   # [768, 192]
            # [pi, dc*192 + c] = wv_c[128*dc + pi, c]
            wvp = np.ascontiguousarray(
                wv_c.reshape(6, P, 192).transpose(1, 0, 2).reshape(P, 6 * 192)
            ).astype(f16)
            wp01 = np.concatenate(
                [Wproj[HD * hs[0] : HD * hs[0] + HD, :],
                 Wproj[HD * hs[1] : HD * hs[1] + HD, :]], axis=0
            ).astype(f16)                              # [128, 768]
            wp2 = np.zeros((P, D), f16)                # [128, 768], rows 64+ zero
            wp2[0:64, :] = Wproj[HD * hs[2] : HD * hs[2] + HD, :].astype(f16)
            bkq = np.zeros((P, 4), np.float32)
            bkq[:, 0] = np.concatenate(
                [bk[HD * hs[0] : HD * hs[0] + HD], bk[HD * hs[1] : HD * hs[1] + HD]]
            )
            bkq[:, 1] = np.concatenate(
                [bq[HD * hs[0] : HD * hs[0] + HD], bq[HD * hs[1] : HD * hs[1] + HD]]
            )
            bkq[0:64, 2] = bk[HD * hs[2] : HD * hs[2] + HD]
            bkq[64:128, 3] = bq[HD * hs[2] : HD * hs[2] + HD]
            in_maps.append(
                dict(xTp=xTp, wkqp=wkqp, wvp=wvp, wp01=wp01, wp2=wp2,
                     bkq=bkq, ident=ident, btri=btri)
            )
    return in_maps, out_bias


def kernel(x, Wkqv, bkqv, Wproj, bproj):
    global _compiled, last_exec_time_ns, last_results
    import concourse.bass_utils as bass_utils

    x = np.asarray(x, np.float32)
    Wkqv = np.asarray(Wkqv, np.float32)
    bkqv = np.asarray(bkqv, np.float32)
    Wproj = np.asarray(Wproj, np.float32)
    bproj = np.asarray(bproj, np.float32)

    if _compiled is None:
        _compiled = _build()
    nc = _compiled

    in_maps, out_bias = _host_prep(x, Wkqv, bkqv, Wproj, bproj)

    trace = os.environ.get("BASS_KERNEL_TRACE", "0") == "1"
    res = bass_utils.run_bass_kernel_spmd(
        nc, in_maps, core_ids=list(range(NCORES)), trace=trace
    )
    last_exec_time_ns = res.exec_time_ns
    last_results = res

    out = np.zeros((B, N, D), np.float32)
    for b in range(B):
        acc = np.zeros((D, N), np.float32)
        for g in range(NG):
            acc += res.results[b * NG + g]["yT"].reshape(D, N).astype(np.float32)
        out[b] = acc.T + out_bias
    return out
